# revision 2
# baseline (speedup 1.0000x reference)
"""CrossAttentionBlock on 8 trn2 NeuronCores — fp8 DoubleRow + mask compaction.

Sharding (per the hint): data parallel over batch B=2, tensor parallel over
heads (16 heads -> 4 groups of 4). Core c = b*4 + hg.

Key ideas vs the fp32r baseline:
  - The kv mask (~50% density) is applied by HOST-side compaction: valid kv
    rows are gathered per batch and padded to SKV_C (multiple of 256). All
    scores/exp/AV work halves. Padding rows are killed by a mask column in
    the V operand (denominator) and a mask multiply on V (numerator).
  - Everything on the attention path runs in fp8e4m3 with DoubleRow matmuls
    (2 k-tiles per instruction at 0.5 cycles/row = 4x fp32r MAC rate):
      * q/k/v projections pair e-tiles (2j, 2j+1) from the natural layouts.
      * scores pair (kT, zeros) on the stationary side and broadcast the
        moving qT pair with a stride-0 AP — halves score cost without
        re-laying-out d=64.
      * AV pairs consecutive kv-tiles; v is stored [128, 4h, NT, 80] (80B
        stride satisfies the DoubleRow step%16 ISA rule) with the mask in
        column 64 so row 64 of the AV accumulator is the softmax denominator.
  - exp(score/8 - 4.5) on ACT writes fp8 directly; the -4.5 shift keeps
    e^s inside fp8 range (scores reach ~9.7) and cancels in the softmax.
  - attnT output is fp8 scaled x16 (via a 16-valued ones vector in the
    denominator-broadcast matmul); Wo is host-scaled x32 and the phase-B
    evacuation descales by 1/512. Residual + LayerNorm stay fp32.

Phase A is ACT(exp)-bound (~75us); PE ~37us, DMA ~13us, DVE ~33us.
Phase B: fp8 DoubleRow out-projection + fp32 LN over 256 rows/core.
"""

import numpy as np
import ml_dtypes
from contextlib import ExitStack

import concourse.bacc as bacc
import concourse.tile as tile
import concourse.mybir as mybir
from concourse.bass_utils import run_bass_kernel_spmd

F32 = mybir.dt.float32
F32R = mybir.dt.float32r
F8 = mybir.dt.float8e4
NPF8 = ml_dtypes.float8_e4m3
AF = mybir.ActivationFunctionType
ALU = mybir.AluOpType
AX = mybir.AxisListType
DR = mybir.MatmulPerfMode.DoubleRow

B, SQ, SKV, E = 2, 1024, 4096, 1024
H, D = 16, 64
HG = 4                 # heads per core
HD = HG * D            # 256
P = 128
NE = E // P            # 8
LN_EPS = 1e-5
SCALE = 1.0 / np.sqrt(D)
EXPC = 4.5             # exp shift: ex = exp(s*SCALE - EXPC), cancels in softmax
ASC = 16.0             # attnT output scale (folded into 1/den broadcast)
WSC = 32.0             # host-side Wo scale
VPAD = 80              # per-(head, kv-tile) v stride in bytes (mult of 16)

_CACHE = {}


def _chunks(skv_c):
    out, s = [], 0
    while s < skv_c:
        w = min(512, skv_c - s)
        out.append((s, w))
        s += w
    return out


def _build_phase_a(skv_c):
    NT = skv_c // P            # kv tiles (even: skv_c % 256 == 0)
    NPAIR = NT // 2
    nc = bacc.Bacc("TRN2", target_bir_lowering=False, debug=False, num_devices=8)

    qT_d = nc.dram_tensor("qT8", [P, NE, SQ], F8, kind="ExternalInput")
    kvT_d = nc.dram_tensor("kvT8", [P, NE, skv_c], F8, kind="ExternalInput")
    wq_d = nc.dram_tensor("wq8", [P, NE, HD], F8, kind="ExternalInput")
    wk_d = nc.dram_tensor("wk8", [P, NE, HD], F8, kind="ExternalInput")
    wv_d = nc.dram_tensor("wv8", [P, NE, HD], F8, kind="ExternalInput")
    bq_d = nc.dram_tensor("bq", [P, 2], F32, kind="ExternalInput")
    bk_d = nc.dram_tensor("bk", [P, 2], F32, kind="ExternalInput")
    bv_d = nc.dram_tensor("bv", [1, HD], F32R, kind="ExternalInput")
    mask_d = nc.dram_tensor("mask01", [P, NT], F32, kind="ExternalInput")
    attnT_d = nc.dram_tensor("attnT8", [HD, SQ], F8, kind="ExternalOutput")

    with tile.TileContext(nc) as tc, ExitStack() as ctx:
        const = ctx.enter_context(tc.tile_pool(name="const", bufs=1))

        wq_sb = const.tile([P, NE, HD], F8)
        nc.sync.dma_start(wq_sb[:], wq_d.ap())
        wk_sb = const.tile([P, NE, HD], F8)
        nc.sync.dma_start(wk_sb[:], wk_d.ap())
        wv_sb = const.tile([P, NE, HD], F8)
        nc.sync.dma_start(wv_sb[:], wv_d.ap())
        bq_sb = const.tile([P, 2], F32)
        nc.sync.dma_start(bq_sb[:], bq_d.ap())
        bk_sb = const.tile([P, 2], F32)
        nc.sync.dma_start(bk_sb[:], bk_d.ap())
        bv_sb = const.tile([1, HD], F32R)
        nc.sync.dma_start(bv_sb[:], bv_d.ap())
        mask_sb = const.tile([P, NT], F32)
        nc.sync.dma_start(mask_sb[:], mask_d.ap())
        qch_sb = const.tile([P, NE, SQ], F8)
        nc.sync.dma_start(qch_sb[:], qT_d.ap())
        kvch_sb = const.tile([P, NE, skv_c], F8)
        chunks = _chunks(skv_c)
        for (s, w) in chunks:
            nc.sync.dma_start(kvch_sb[:, :, s:s + w], kvT_d.ap()[:, :, s:s + w])

        nbias = const.tile([P, 1], F32)
        nc.any.memset(nbias[:], -EXPC)
        ones32 = const.tile([1, P], F32)
        nc.any.memset(ones32[:], 1.0)
        ones1 = const.tile([1, P], F32R)
        nc.vector.tensor_copy(ones1[:], ones32[:])
        o16_32 = const.tile([1, D], F32)
        nc.any.memset(o16_32[:], ASC)
        ones16 = const.tile([1, D], F32R)
        nc.vector.tensor_copy(ones16[:], o16_32[:])

        qT8 = const.tile([P, 2, SQ], F8)          # projected q, [d-part, m, q]
        kT8 = const.tile([P, 2, 2, skv_c], F8)    # [d-part, m, (real|zero), kv]
        nc.any.memset(kT8[:, :, 1, :], 0.0)
        v8 = const.tile([P, HG, NT, VPAD], F8)    # [kv-part, h, kv-tile, 64 v + mask + pad]
        for h in range(HG):
            nc.vector.tensor_copy(v8[:, h, :, D], mask_sb[:])
        attnT_sb = const.tile([P, 2, SQ], F8)

        sc_ps = ctx.enter_context(tc.tile_pool(name="scps", bufs=2, space="PSUM"))
        pv_ps = ctx.enter_context(tc.tile_pool(name="pvps", bufs=1, space="PSUM"))
        pj_ps = ctx.enter_context(tc.tile_pool(name="pjps", bufs=1, space="PSUM"))
        ex_pool = ctx.enter_context(tc.tile_pool(name="expool", bufs=3))
        sm_pool = ctx.enter_context(tc.tile_pool(name="smpool", bufs=2))

        def q_proj():
            for m in range(2):
                for qc in range(2):
                    ps = pj_ps.tile([P, 512], F32, tag="qk", name=f"qps{m}{qc}")
                    for jp in range(NE // 2):
                        nc.tensor.matmul(
                            ps[:], wq_sb[:, 2 * jp:2 * jp + 2, m * P:(m + 1) * P],
                            qch_sb[:, 2 * jp:2 * jp + 2, qc * 512:(qc + 1) * 512],
                            start=(jp == 0), stop=(jp == NE // 2 - 1), perf_mode=DR)
                    nc.vector.tensor_scalar(qT8[:, m, qc * 512:(qc + 1) * 512],
                                            ps[:], bq_sb[:, m:m + 1], None, op0=ALU.add)

        def k_proj(s, w):
            for m in range(2):
                ps = pj_ps.tile([P, 512], F32, tag="qk", name=f"kps{m}_{s}")
                for jp in range(NE // 2):
                    nc.tensor.matmul(
                        ps[:, 0:w], wk_sb[:, 2 * jp:2 * jp + 2, m * P:(m + 1) * P],
                        kvch_sb[:, 2 * jp:2 * jp + 2, s:s + w],
                        start=(jp == 0), stop=(jp == NE // 2 - 1), perf_mode=DR)
                nc.vector.tensor_scalar(kT8[:, m, 0, s:s + w], ps[:, 0:w],
                                        bk_sb[:, m:m + 1], None, op0=ALU.add)

        def v_proj(t):
            ps = pj_ps.tile([P, HD], F32, tag="v", name=f"vps{t}")
            for jp in range(NE // 2):
                nc.tensor.matmul(
                    ps[:], kvch_sb[:, 2 * jp:2 * jp + 2, t * P:(t + 1) * P],
                    wv_sb[:, 2 * jp:2 * jp + 2, :],
                    start=(jp == 0), stop=False, perf_mode=DR)
            nc.tensor.matmul(ps[:], ones1[:], bv_sb[:], start=False, stop=True)
            nc.vector.tensor_scalar(
                v8[:, :, t, 0:D],
                ps[:].rearrange("p (h d) -> p h d", d=D),
                mask_sb[:, t:t + 1], None, op0=ALU.mult)

        def score_exp(h, t, exu):
            m, hh = divmod(h, 2)
            ps = sc_ps.tile([P, SQ], F32, tag="sc", name=f"s{h}_{t}")
            kp = kT8[hh * D:(hh + 1) * D, m, :, t * P:(t + 1) * P]
            for qh in range(2):
                qp = qT8[hh * D:(hh + 1) * D, m, qh * 512:(qh + 1) * 512] \
                    .unsqueeze(1).broadcast_to([D, 2, 512])
                nc.tensor.matmul(ps[:, qh * 512:(qh + 1) * 512], kp, qp,
                                 start=True, stop=True, perf_mode=DR)
            nc.scalar.activation(exu[:, t % 2, :], ps[:], AF.Exp,
                                 scale=float(SCALE), bias=nbias[:])

        def av(h, u, exu, pv):
            for qh in range(2):
                nc.tensor.matmul(
                    pv[qh][:], v8[:, h, 2 * u:2 * u + 2, 0:D + 1],
                    exu[:, :, qh * 512:(qh + 1) * 512],
                    start=(u == 0), stop=(u == NPAIR - 1), perf_mode=DR)

        def norm(h, pv):
            m, hh = divmod(h, 2)
            for qh in range(2):
                den = sm_pool.tile([1, 512], F32, tag="den", name=f"den{h}{qh}")
                nc.vector.tensor_copy(den[:], pv[qh][D:D + 1, :])
                rec = sm_pool.tile([1, 512], F32R, tag="rec", name=f"rec{h}{qh}")
                with nc.allow_low_precision(reason="recip feeds f32r matmul"):
                    nc.vector.reciprocal(rec[:], den[:])
                raw = sm_pool.tile([D, 512], F32, tag="raw", name=f"raw{h}{qh}")
                nc.vector.tensor_copy(raw[:], pv[qh][0:D, :])
                bc = pj_ps.tile([P, 512], F32, tag="qk", name=f"bc{h}{qh}")
                nc.tensor.matmul(bc[0:D, :], ones16[:], rec[:], start=True, stop=True)
                nc.vector.tensor_tensor(
                    attnT_sb[hh * D:(hh + 1) * D, m, qh * 512:(qh + 1) * 512],
                    raw[:], bc[0:D, :], op=ALU.mult)

        def new_pv(h):
            return [pv_ps.tile([D + 1, 512], F32, tag=f"pv{qh}", name=f"pv{h}_{qh}")
                    for qh in range(2)]

        # head 0 sweeps behind the projection chunks; heads 1-3 sweep after
        q_proj()
        pv0 = new_pv(0)
        ex0 = None
        for ci, (s, w) in enumerate(chunks):
            k_proj(s, w)
            t0 = s // P
            for t in range(t0, t0 + w // P):
                v_proj(t)
            for t in range(t0, t0 + w // P):
                if t % 2 == 0:
                    ex0 = ex_pool.tile([P, 2, SQ], F8, tag="ex", name=f"ex0_{t}")
                score_exp(0, t, ex0)
                if t % 2 == 1:
                    av(0, t // 2, ex0, pv0)
        norm(0, pv0)

        for h in range(1, HG):
            pv = new_pv(h)
            exu = None
            for t in range(NT):
                if t % 2 == 0:
                    exu = ex_pool.tile([P, 2, SQ], F8, tag="ex", name=f"ex{h}_{t}")
                score_exp(h, t, exu)
                if t % 2 == 1:
                    av(h, t // 2, exu, pv)
            norm(h, pv)

        nc.sync.dma_start(attnT_d.ap().rearrange("(m p) q -> p m q", p=P), attnT_sb[:])

    nc.compile()
    return nc


def _build_phase_b():
    R = 2 * P   # 256 rows per core
    nc = bacc.Bacc("TRN2", target_bir_lowering=False, debug=False, num_devices=8)

    aT_d = nc.dram_tensor("aT8", [P, NE, R], F8, kind="ExternalInput")
    wo_d = nc.dram_tensor("wo8", [P, NE, E], F8, kind="ExternalInput")
    qn_d = nc.dram_tensor("qn", [R, E], F32, kind="ExternalInput")
    bo_d = nc.dram_tensor("bo512", [1, E], F32R, kind="ExternalInput")
    gam_d = nc.dram_tensor("gam", [1, E], F32R, kind="ExternalInput")
    bet_d = nc.dram_tensor("bet", [1, E], F32R, kind="ExternalInput")
    y_d = nc.dram_tensor("y", [R, E], F32, kind="ExternalOutput")

    with tile.TileContext(nc) as tc, ExitStack() as ctx:
        const = ctx.enter_context(tc.tile_pool(name="const", bufs=1))
        aT_sb = const.tile([P, NE, R], F8)
        nc.sync.dma_start(aT_sb[:], aT_d.ap())
        wo_sb = const.tile([P, NE, E], F8)
        nc.sync.dma_start(wo_sb[:], wo_d.ap())
        qn_sb = const.tile([P, 2, E], F32)
        for mt in range(2):
            nc.sync.dma_start(qn_sb[:, mt, :],
                              qn_d.ap().rearrange("(m p) e -> p m e", p=P)[:, mt, :])
        bo_sb = const.tile([1, E], F32R)
        nc.sync.dma_start(bo_sb[:], bo_d.ap())
        gam_sb = const.tile([1, E], F32R)
        nc.sync.dma_start(gam_sb[:], gam_d.ap())
        bet_sb = const.tile([1, E], F32R)
        nc.sync.dma_start(bet_sb[:], bet_d.ap())
        ones32_sb = const.tile([1, P], F32)
        nc.any.memset(ones32_sb[:], 1.0)
        ones_sb = const.tile([1, P], F32R)
        nc.vector.tensor_copy(ones_sb[:], ones32_sb[:])

        gam_bc = const.tile([P, E], F32)
        bet_bc = const.tile([P, E], F32)

        ps_pool = ctx.enter_context(tc.tile_pool(name="ps", bufs=2, space="PSUM"))
        gb_ps = ctx.enter_context(tc.tile_pool(name="gbps", bufs=2, space="PSUM"))
        sbp = ctx.enter_context(tc.tile_pool(name="sbp", bufs=2))

        ps_tiles = {}
        for mt in range(2):
            ps_tiles[mt] = ps_pool.tile([P, E], F32, tag="o", name=f"o{mt}")
            for nh in range(2):
                for jp in range(NE // 2):
                    nc.tensor.matmul(ps_tiles[mt][:, nh * 512:(nh + 1) * 512],
                                     aT_sb[:, 2 * jp:2 * jp + 2, mt * P:(mt + 1) * P],
                                     wo_sb[:, 2 * jp:2 * jp + 2, nh * 512:(nh + 1) * 512],
                                     start=(jp == 0), stop=False, perf_mode=DR)
                nc.tensor.matmul(ps_tiles[mt][:, nh * 512:(nh + 1) * 512], ones_sb[:],
                                 bo_sb[:, nh * 512:(nh + 1) * 512],
                                 start=False, stop=True)

        # broadcast gamma/beta rows to all 128 partitions via K=1 matmuls
        for half in range(2):
            cs = slice(half * 512, (half + 1) * 512)
            psg = gb_ps.tile([P, 512], F32, tag="gb", name=f"gbg{half}")
            nc.tensor.matmul(psg[:], ones_sb[:], gam_sb[:, cs], start=True, stop=True)
            nc.scalar.copy(gam_bc[:, cs], psg[:])
            psb = gb_ps.tile([P, 512], F32, tag="gb", name=f"gbb{half}")
            nc.tensor.matmul(psb[:], ones_sb[:], bet_sb[:, cs], start=True, stop=True)
            nc.scalar.copy(bet_bc[:, cs], psb[:])

        # evac (descale 1/512) + residual + LayerNorm
        for mt in range(2):
            o32 = sbp.tile([P, E], F32, tag="o32", name=f"o32{mt}")
            nc.scalar.activation(o32[:], ps_tiles[mt][:], AF.Copy,
                                 scale=1.0 / (ASC * WSC))
            x = sbp.tile([P, E], F32, tag="x", name=f"x{mt}")
            nc.vector.tensor_tensor(x[:], o32[:], qn_sb[:, mt, :], op=ALU.add)
            s1 = sbp.tile([P, 1], F32, tag="s1", name=f"s1{mt}")
            nc.vector.reduce_sum(s1[:], x[:], axis=AX.X)
            sq = sbp.tile([P, E], F32, tag="sq", name=f"sq{mt}")
            ssq = sbp.tile([P, 1], F32, tag="ssq", name=f"ssq{mt}")
            nc.scalar.activation(sq[:], x[:], AF.Square, accum_out=ssq[:])
            nm = sbp.tile([P, 1], F32, tag="nm", name=f"nm{mt}")
            nc.vector.tensor_scalar(nm[:], s1[:], -1.0 / E, None, op0=ALU.mult)
            m2 = sbp.tile([P, 1], F32, tag="m2", name=f"m2{mt}")
            nc.vector.tensor_tensor(m2[:], nm[:], nm[:], op=ALU.mult)
            var = sbp.tile([P, 1], F32, tag="var", name=f"var{mt}")
            nc.vector.tensor_scalar(var[:], ssq[:], 1.0 / E, LN_EPS, op0=ALU.mult, op1=ALU.add)
            nc.vector.tensor_tensor(var[:], var[:], m2[:], op=ALU.subtract)
            rv = sbp.tile([P, 1], F32, tag="rv", name=f"rv{mt}")
            nc.vector.reciprocal(rv[:], var[:])
            rstd = sbp.tile([P, 1], F32, tag="rstd", name=f"rstd{mt}")
            nc.scalar.activation(rstd[:], rv[:], AF.Sqrt)
            yn = sbp.tile([P, E], F32, tag="yn", name=f"yn{mt}")
            nc.vector.tensor_scalar(yn[:], x[:], nm[:], rstd[:], op0=ALU.add, op1=ALU.mult)
            yg = sbp.tile([P, E], F32, tag="yg", name=f"yg{mt}")
            nc.vector.tensor_tensor(yg[:], yn[:], gam_bc[:], op=ALU.mult)
            yb = sbp.tile([P, E], F32, tag="yb", name=f"yb{mt}")
            nc.vector.tensor_tensor(yb[:], yg[:], bet_bc[:], op=ALU.add)
            nc.sync.dma_start(y_d.ap().rearrange("(m p) e -> p m e", p=P)[:, mt, :], yb[:])

    nc.compile()
    return nc


def _get(name, skv_c=None):
    key = (name, skv_c)
    if key not in _CACHE:
        _CACHE[key] = _build_phase_a(skv_c) if name == "a" else _build_phase_b()
    return _CACHE[key]


def _to_jslices(x):
    """[E, N] -> [P, NE, N] with j-slice (e // 128) as a free dim."""
    e, n = x.shape
    return np.ascontiguousarray(x.reshape(NE, P, n).transpose(1, 0, 2))


def kernel(query, key_value, key_value_mask, Wq, bq, Wk, bk, Wv, bv, Wo, bo,
           ln_gamma, ln_beta):
    f = lambda a: np.ascontiguousarray(np.asarray(a, dtype=np.float32))
    f8 = lambda a: np.ascontiguousarray(np.asarray(a, dtype=np.float32).astype(NPF8))
    query, key_value = f(query), f(key_value)
    Wq, Wk, Wv, Wo = f(Wq), f(Wk), f(Wv), f(Wo)
    bq, bk, bv, bo = f(bq), f(bk), f(bv), f(bo)
    ln_gamma, ln_beta = f(ln_gamma), f(ln_beta)
    maskb = np.asarray(key_value_mask) != 0

    # host-side kv compaction (pure gather + zero pad, shared per batch)
    pops = [int(maskb[b].sum()) for b in range(B)]
    skv_c = max(256, int(np.ceil(max(max(pops), 1) / 256.0)) * 256)
    skv_c = min(skv_c, SKV if SKV % 256 == 0 else SKV)
    NT = skv_c // P
    kvT8s, mask01s = [], []
    for b in range(B):
        idx = np.flatnonzero(maskb[b])
        kvc = np.zeros((skv_c, E), np.float32)
        kvc[:len(idx)] = key_value[b][idx]
        mc = np.zeros((skv_c,), np.float32)
        mc[:len(idx)] = 1.0
        kvT8s.append(f8(_to_jslices(kvc.T)))
        mask01s.append(f(mc.reshape(NT, P).T))

    def shufw(w):
        # [256 out-dims, E] -> lhsT j-slices [P, NE, HD]
        return f8(_to_jslices(w.T))

    nc_a = _get("a", skv_c)
    in_maps_a = []
    qT8s = [f8(_to_jslices(query[b].T)) for b in range(B)]
    for c in range(8):
        b, hg = c // 4, c % 4
        sl = slice(hg * HD, (hg + 1) * HD)
        in_maps_a.append({
            "qT8": qT8s[b],
            "kvT8": kvT8s[b],
            "wq8": shufw(Wq[sl]),
            "wk8": shufw(Wk[sl]),
            "wv8": shufw(Wv[sl]),
            "bq": f(bq[sl].reshape(2, P).T),
            "bk": f(bk[sl].reshape(2, P).T),
            "bv": bv[sl].reshape(1, HD),
            "mask01": mask01s[b],
        })
    res_a = run_bass_kernel_spmd(nc_a, in_maps_a, core_ids=list(range(8)))
    # gather: per batch, stack head-group slabs into the full [E, SQ] fp8 attnT
    attnT = [np.concatenate([res_a.results[b * 4 + hg]["attnT8"] for hg in range(4)],
                            axis=0) for b in range(B)]

    nc_b = _get("b")
    wo8 = f8(_to_jslices(Wo.T * WSC))
    bo512 = f(bo * (ASC * WSC)).reshape(1, E)
    gam_r = ln_gamma.reshape(1, E)
    bet_r = ln_beta.reshape(1, E)
    in_maps_b = []
    for c in range(8):
        b, j = c // 4, c % 4
        rs = slice(j * 256, (j + 1) * 256)
        in_maps_b.append({
            "aT8": np.ascontiguousarray(_to_jslices(attnT[b])[:, :, rs]),
            "wo8": wo8,
            "qn": f(query[b, rs, :]),
            "bo512": bo512,
            "gam": gam_r,
            "bet": bet_r,
        })
    res_b = run_bass_kernel_spmd(nc_b, in_maps_b, core_ids=list(range(8)))
    out = np.empty((B, SQ, E), np.float32)
    for c in range(8):
        b, j = c // 4, c % 4
        out[b, j * 256:(j + 1) * 256, :] = res_b.results[c]["y"]
    return out


# revision 8
# speedup vs baseline: 1.0135x; 1.0135x over previous
"""CrossAttentionBlock on 8 trn2 NeuronCores — fp8 DoubleRow + mask compaction.

Sharding (per the hint): data parallel over batch B=2, tensor parallel over
heads (16 heads -> 4 groups of 4). Core c = b*4 + hg.

Key ideas vs the fp32r baseline:
  - The kv mask (~50% density) is applied by HOST-side compaction: valid kv
    rows are gathered per batch and padded to SKV_C (multiple of 256). All
    scores/exp/AV work halves. Padding rows are killed by a mask column in
    the V operand (denominator) and a mask multiply on V (numerator).
  - Everything on the attention path runs in fp8e4m3 with DoubleRow matmuls
    (2 k-tiles per instruction at 0.5 cycles/row = 4x fp32r MAC rate):
      * q/k/v projections pair e-tiles (2j, 2j+1) from the natural layouts.
      * scores pair (kT, zeros) on the stationary side and broadcast the
        moving qT pair with a stride-0 AP — halves score cost without
        re-laying-out d=64.
      * AV pairs consecutive kv-tiles; v is stored [128, 4h, NT, 80] (80B
        stride satisfies the DoubleRow step%16 ISA rule) with the mask in
        column 64 so row 64 of the AV accumulator is the softmax denominator.
  - exp(score/8 - 4.5) on ACT writes fp8 directly; the -4.5 shift keeps
    e^s inside fp8 range (scores reach ~9.7) and cancels in the softmax.
  - attnT output is fp8 scaled x16 (via a 16-valued ones vector in the
    denominator-broadcast matmul); Wo is host-scaled x32 and the phase-B
    evacuation descales by 1/512. Residual + LayerNorm stay fp32.

Phase A is ACT(exp)-bound (~75us); PE ~37us, DMA ~13us, DVE ~33us.
Phase B: fp8 DoubleRow out-projection + fp32 LN over 256 rows/core.
"""

import numpy as np
import ml_dtypes
from contextlib import ExitStack

import concourse.bacc as bacc
import concourse.tile as tile
import concourse.mybir as mybir
from concourse.bass_utils import run_bass_kernel_spmd

F32 = mybir.dt.float32
F32R = mybir.dt.float32r
F8 = mybir.dt.float8e4
NPF8 = ml_dtypes.float8_e4m3
AF = mybir.ActivationFunctionType
ALU = mybir.AluOpType
AX = mybir.AxisListType
DR = mybir.MatmulPerfMode.DoubleRow

B, SQ, SKV, E = 2, 1024, 4096, 1024
H, D = 16, 64
HG = 4                 # heads per core
HD = HG * D            # 256
P = 128
NE = E // P            # 8
LN_EPS = 1e-5
SCALE = 1.0 / np.sqrt(D)
EXPC = 4.5             # exp shift: ex = exp(s*SCALE - EXPC), cancels in softmax
ASC = 16.0             # attnT output scale (folded into 1/den broadcast)
WSC = 32.0             # host-side Wo scale
VPAD = 80              # per-(head, kv-tile) v stride in bytes (mult of 16)

_CACHE = {}


def _chunks(skv_c):
    out, s = [], 0
    while s < skv_c:
        w = min(512, skv_c - s)
        out.append((s, w))
        s += w
    return out


def _build_phase_a(skv_c):
    NT = skv_c // P            # kv tiles (even: skv_c % 256 == 0)
    NPAIR = NT // 2
    nc = bacc.Bacc("TRN2", target_bir_lowering=False, debug=False, num_devices=8)

    qT_d = nc.dram_tensor("qT8", [P, NE, SQ], F8, kind="ExternalInput")
    kvT_d = nc.dram_tensor("kvT8", [P, NE, skv_c], F8, kind="ExternalInput")
    wq_d = nc.dram_tensor("wq8", [P, NE, HD], F8, kind="ExternalInput")
    wk_d = nc.dram_tensor("wk8", [P, NE, HD], F8, kind="ExternalInput")
    wv_d = nc.dram_tensor("wv8", [P, NE, HD], F8, kind="ExternalInput")
    bq_d = nc.dram_tensor("bq", [P, 2], F32, kind="ExternalInput")
    bk_d = nc.dram_tensor("bk", [P, 2], F32, kind="ExternalInput")
    bv_d = nc.dram_tensor("bv", [1, HD], F32R, kind="ExternalInput")
    mask_d = nc.dram_tensor("mask01", [P, NT], F32, kind="ExternalInput")
    attnT_d = nc.dram_tensor("attnT8", [HD, SQ], F8, kind="ExternalOutput")

    with tile.TileContext(nc) as tc, ExitStack() as ctx:
        const = ctx.enter_context(tc.tile_pool(name="const", bufs=1))

        # DMA order is the phase-A startup critical path: tiny scalars, then
        # wk + kv chunk 0 (gate k_proj), then qT + wq (gate q_proj -> first
        # exp), then wv and the remaining kv chunks.
        bq_sb = const.tile([P, 2], F32)
        nc.sync.dma_start(bq_sb[:], bq_d.ap())
        bk_sb = const.tile([P, 2], F32)
        nc.sync.dma_start(bk_sb[:], bk_d.ap())
        bv_sb = const.tile([1, HD], F32R)
        nc.sync.dma_start(bv_sb[:], bv_d.ap())
        mask_sb = const.tile([P, NT], F32)
        nc.sync.dma_start(mask_sb[:], mask_d.ap())
        chunks = _chunks(skv_c)
        wk_sb = const.tile([P, NE, HD], F8)
        nc.sync.dma_start(wk_sb[:], wk_d.ap())
        kvch_sb = const.tile([P, NE, skv_c], F8)
        s0, w0 = chunks[0]
        nc.sync.dma_start(kvch_sb[:, :, s0:s0 + w0], kvT_d.ap()[:, :, s0:s0 + w0])
        qch_sb = const.tile([P, NE, SQ], F8)
        nc.sync.dma_start(qch_sb[:], qT_d.ap())
        wq_sb = const.tile([P, NE, HD], F8)
        nc.sync.dma_start(wq_sb[:], wq_d.ap())
        wv_sb = const.tile([P, NE, HD], F8)
        nc.sync.dma_start(wv_sb[:], wv_d.ap())
        for (s, w) in chunks[1:]:
            nc.sync.dma_start(kvch_sb[:, :, s:s + w], kvT_d.ap()[:, :, s:s + w])

        nbias = const.tile([P, 1], F32)
        nc.any.memset(nbias[:], -EXPC)
        ones32 = const.tile([1, P], F32)
        nc.any.memset(ones32[:], 1.0)
        ones1 = const.tile([1, P], F32R)
        nc.vector.tensor_copy(ones1[:], ones32[:])
        o16_32 = const.tile([1, D], F32)
        nc.any.memset(o16_32[:], ASC)
        ones16 = const.tile([1, D], F32R)
        nc.vector.tensor_copy(ones16[:], o16_32[:])

        qT8 = const.tile([P, 2, SQ], F8)          # projected q, [d-part, m, q]
        kT8 = const.tile([P, 2, 2, skv_c], F8)    # [d-part, m, (real|zero), kv]
        nc.any.memset(kT8[:, :, 1, :], 0.0)
        v8 = const.tile([P, HG, NT, VPAD], F8)    # [kv-part, h, kv-tile, 64 v + mask + pad]
        for h in range(HG):
            nc.vector.tensor_copy(v8[:, h, :, D], mask_sb[:])
        attnT_sb = const.tile([P, 2, SQ], F8)

        sc_ps = ctx.enter_context(tc.tile_pool(name="scps", bufs=2, space="PSUM"))
        pv_ps = ctx.enter_context(tc.tile_pool(name="pvps", bufs=1, space="PSUM"))
        pj_ps = ctx.enter_context(tc.tile_pool(name="pjps", bufs=2, space="PSUM"))
        ex_pool = ctx.enter_context(tc.tile_pool(name="expool", bufs=3))
        sm_pool = ctx.enter_context(tc.tile_pool(name="smpool", bufs=2))

        def q_proj():
            for m in range(2):
                for qc in range(2):
                    ps = pj_ps.tile([P, 512], F32, tag="qk", name=f"qps{m}{qc}")
                    for jp in range(NE // 2):
                        nc.tensor.matmul(
                            ps[:], wq_sb[:, 2 * jp:2 * jp + 2, m * P:(m + 1) * P],
                            qch_sb[:, 2 * jp:2 * jp + 2, qc * 512:(qc + 1) * 512],
                            start=(jp == 0), stop=(jp == NE // 2 - 1), perf_mode=DR)
                    nc.vector.tensor_scalar(qT8[:, m, qc * 512:(qc + 1) * 512],
                                            ps[:], bq_sb[:, m:m + 1], None, op0=ALU.add)

        def k_proj(s, w):
            for m in range(2):
                ps = pj_ps.tile([P, 512], F32, tag="qk", name=f"kps{m}_{s}")
                for jp in range(NE // 2):
                    nc.tensor.matmul(
                        ps[:, 0:w], wk_sb[:, 2 * jp:2 * jp + 2, m * P:(m + 1) * P],
                        kvch_sb[:, 2 * jp:2 * jp + 2, s:s + w],
                        start=(jp == 0), stop=(jp == NE // 2 - 1), perf_mode=DR)
                nc.vector.tensor_scalar(kT8[:, m, 0, s:s + w], ps[:, 0:w],
                                        bk_sb[:, m:m + 1], None, op0=ALU.add)

        def v_proj(t):
            psf = pj_ps.tile([P, 512], F32, tag="qk", name=f"vps{t}")
            ps = psf[:, 0:HD]
            for jp in range(NE // 2):
                nc.tensor.matmul(
                    ps, kvch_sb[:, 2 * jp:2 * jp + 2, t * P:(t + 1) * P],
                    wv_sb[:, 2 * jp:2 * jp + 2, :],
                    start=(jp == 0), stop=False, perf_mode=DR)
            nc.tensor.matmul(ps, ones1[:], bv_sb[:], start=False, stop=True)
            nc.vector.tensor_scalar(
                v8[:, :, t, 0:D],
                ps.rearrange("p (h d) -> p h d", d=D),
                mask_sb[:, t:t + 1], None, op0=ALU.mult)

        def score_exp(h, t, exu):
            m, hh = divmod(h, 2)
            ps = sc_ps.tile([P, SQ], F32, tag="sc", name=f"s{h}_{t}")
            kp = kT8[hh * D:(hh + 1) * D, m, :, t * P:(t + 1) * P]
            for qh in range(2):
                qp = qT8[hh * D:(hh + 1) * D, m, qh * 512:(qh + 1) * 512] \
                    .unsqueeze(1).broadcast_to([D, 2, 512])
                nc.tensor.matmul(ps[:, qh * 512:(qh + 1) * 512], kp, qp,
                                 start=True, stop=True, perf_mode=DR)
            nc.scalar.activation(exu[:, t % 2, :], ps[:], AF.Exp,
                                 scale=float(SCALE), bias=nbias[:])

        def av(h, u, exu, pv):
            for qh in range(2):
                nc.tensor.matmul(
                    pv[qh][:], v8[:, h, 2 * u:2 * u + 2, 0:D + 1],
                    exu[:, :, qh * 512:(qh + 1) * 512],
                    start=(u == 0), stop=(u == NPAIR - 1), perf_mode=DR)

        def norm(h, pv):
            m, hh = divmod(h, 2)
            for qh in range(2):
                den = sm_pool.tile([1, 512], F32, tag="den", name=f"den{h}{qh}")
                nc.vector.tensor_copy(den[:], pv[qh][D:D + 1, :])
                rec = sm_pool.tile([1, 512], F32R, tag="rec", name=f"rec{h}{qh}")
                with nc.allow_low_precision(reason="recip feeds f32r matmul"):
                    nc.vector.reciprocal(rec[:], den[:])
                raw = sm_pool.tile([D, 512], F32, tag="raw", name=f"raw{h}{qh}")
                nc.vector.tensor_copy(raw[:], pv[qh][0:D, :])
                bc = pj_ps.tile([P, 512], F32, tag="qk", name=f"bc{h}{qh}")
                nc.tensor.matmul(bc[0:D, :], ones16[:], rec[:], start=True, stop=True)
                nc.vector.tensor_tensor(
                    attnT_sb[hh * D:(hh + 1) * D, m, qh * 512:(qh + 1) * 512],
                    raw[:], bc[0:D, :], op=ALU.mult)

        def new_pv(h):
            return [pv_ps.tile([D + 1, 512], F32, tag=f"pv{qh}", name=f"pv{h}_{qh}")
                    for qh in range(2)]

        # head 0 sweeps behind the projection chunks; heads 1-3 sweep after.
        # v_proj(t) and score_exp(t) interleave so PE evacuations overlap the
        # next tile's matmuls and the ACT exp stream starts ASAP.
        q_proj()
        pv0 = new_pv(0)
        ex0 = None
        for ci, (s, w) in enumerate(chunks):
            k_proj(s, w)
            t0 = s // P
            for t in range(t0, t0 + w // P):
                v_proj(t)
                if t % 2 == 0:
                    ex0 = ex_pool.tile([P, 2, SQ], F8, tag="ex", name=f"ex0_{t}")
                score_exp(0, t, ex0)
                if t % 2 == 1:
                    av(0, t // 2, ex0, pv0)
        norm(0, pv0)

        for h in range(1, HG):
            pv = new_pv(h)
            exu = None
            for t in range(NT):
                if t % 2 == 0:
                    exu = ex_pool.tile([P, 2, SQ], F8, tag="ex", name=f"ex{h}_{t}")
                score_exp(h, t, exu)
                if t % 2 == 1:
                    av(h, t // 2, exu, pv)
            norm(h, pv)

        nc.sync.dma_start(attnT_d.ap().rearrange("(m p) q -> p m q", p=P), attnT_sb[:])

    nc.compile()
    return nc


def _build_phase_b():
    R = 2 * P   # 256 rows per core
    nc = bacc.Bacc("TRN2", target_bir_lowering=False, debug=False, num_devices=8)

    aT_d = nc.dram_tensor("aT8", [P, NE, R], F8, kind="ExternalInput")
    wo_d = nc.dram_tensor("wo8", [P, NE, E], F8, kind="ExternalInput")
    qn_d = nc.dram_tensor("qn", [R, E], F32, kind="ExternalInput")
    bo_d = nc.dram_tensor("bo512", [1, E], F32R, kind="ExternalInput")
    gam_d = nc.dram_tensor("gam", [1, E], F32R, kind="ExternalInput")
    bet_d = nc.dram_tensor("bet", [1, E], F32R, kind="ExternalInput")
    y_d = nc.dram_tensor("y", [R, E], F32, kind="ExternalOutput")

    with tile.TileContext(nc) as tc, ExitStack() as ctx:
        const = ctx.enter_context(tc.tile_pool(name="const", bufs=1))
        aT_sb = const.tile([P, NE, R], F8)
        nc.sync.dma_start(aT_sb[:], aT_d.ap())
        wo_sb = const.tile([P, NE, E], F8)
        nc.sync.dma_start(wo_sb[:], wo_d.ap())
        qn_sb = const.tile([P, 2, E], F32)
        nc.sync.dma_start(qn_sb[:], qn_d.ap().rearrange("(m p) e -> p m e", p=P))
        bo_sb = const.tile([1, E], F32R)
        nc.sync.dma_start(bo_sb[:], bo_d.ap())
        gam_sb = const.tile([1, E], F32R)
        nc.sync.dma_start(gam_sb[:], gam_d.ap())
        bet_sb = const.tile([1, E], F32R)
        nc.sync.dma_start(bet_sb[:], bet_d.ap())
        ones32_sb = const.tile([1, P], F32)
        nc.any.memset(ones32_sb[:], 1.0)
        ones_sb = const.tile([1, P], F32R)
        nc.vector.tensor_copy(ones_sb[:], ones32_sb[:])

        gam_bc = const.tile([P, E], F32)
        bet_bc = const.tile([P, E], F32)

        ps_pool = ctx.enter_context(tc.tile_pool(name="ps", bufs=2, space="PSUM"))
        gb_ps = ctx.enter_context(tc.tile_pool(name="gbps", bufs=2, space="PSUM"))
        sbp = ctx.enter_context(tc.tile_pool(name="sbp", bufs=2))

        ps_tiles = {}
        for mt in range(2):
            ps_tiles[mt] = ps_pool.tile([P, E], F32, tag="o", name=f"o{mt}")
            for nh in range(2):
                for jp in range(NE // 2):
                    nc.tensor.matmul(ps_tiles[mt][:, nh * 512:(nh + 1) * 512],
                                     aT_sb[:, 2 * jp:2 * jp + 2, mt * P:(mt + 1) * P],
                                     wo_sb[:, 2 * jp:2 * jp + 2, nh * 512:(nh + 1) * 512],
                                     start=(jp == 0), stop=False, perf_mode=DR)
                nc.tensor.matmul(ps_tiles[mt][:, nh * 512:(nh + 1) * 512], ones_sb[:],
                                 bo_sb[:, nh * 512:(nh + 1) * 512],
                                 start=False, stop=True)

        # broadcast gamma/beta rows to all 128 partitions via K=1 matmuls
        # (evacuated on DVE to keep ACT free for the main chain)
        for half in range(2):
            cs = slice(half * 512, (half + 1) * 512)
            psg = gb_ps.tile([P, 512], F32, tag="gb", name=f"gbg{half}")
            nc.tensor.matmul(psg[:], ones_sb[:], gam_sb[:, cs], start=True, stop=True)
            nc.vector.tensor_copy(gam_bc[:, cs], psg[:])
            psb = gb_ps.tile([P, 512], F32, tag="gb", name=f"gbb{half}")
            nc.tensor.matmul(psb[:], ones_sb[:], bet_sb[:, cs], start=True, stop=True)
            nc.vector.tensor_copy(bet_bc[:, cs], psb[:])

        # row sums of the residual input, off the critical chain
        sqn = {}
        for mt in range(2):
            sqn[mt] = sbp.tile([P, 1], F32, tag="sqn", name=f"sqn{mt}")
            nc.vector.reduce_sum(sqn[mt][:], qn_sb[:, mt, :], axis=AX.X)

        # evac (descale 1/512, accumulate row sums) + residual + LayerNorm
        for mt in range(2):
            o32 = sbp.tile([P, E], F32, tag="o32", name=f"o32{mt}")
            so = sbp.tile([P, 1], F32, tag="so", name=f"so{mt}")
            nc.scalar.activation(o32[:], ps_tiles[mt][:], AF.Copy,
                                 scale=1.0 / (ASC * WSC), accum_out=so[:])
            x = sbp.tile([P, E], F32, tag="x", name=f"x{mt}")
            nc.vector.tensor_tensor(x[:], o32[:], qn_sb[:, mt, :], op=ALU.add)
            s1 = sbp.tile([P, 1], F32, tag="s1", name=f"s1{mt}")
            nc.vector.tensor_tensor(s1[:], so[:], sqn[mt][:], op=ALU.add)
            sq = sbp.tile([P, E], F32, tag="sq", name=f"sq{mt}")
            ssq = sbp.tile([P, 1], F32, tag="ssq", name=f"ssq{mt}")
            nc.scalar.activation(sq[:], x[:], AF.Square, accum_out=ssq[:])
            nm = sbp.tile([P, 1], F32, tag="nm", name=f"nm{mt}")
            nc.vector.tensor_scalar(nm[:], s1[:], -1.0 / E, None, op0=ALU.mult)
            m2 = sbp.tile([P, 1], F32, tag="m2", name=f"m2{mt}")
            nc.vector.tensor_tensor(m2[:], nm[:], nm[:], op=ALU.mult)
            var = sbp.tile([P, 1], F32, tag="var", name=f"var{mt}")
            nc.vector.tensor_scalar(var[:], ssq[:], 1.0 / E, LN_EPS, op0=ALU.mult, op1=ALU.add)
            nc.vector.tensor_tensor(var[:], var[:], m2[:], op=ALU.subtract)
            rv = sbp.tile([P, 1], F32, tag="rv", name=f"rv{mt}")
            nc.vector.reciprocal(rv[:], var[:])
            rstd = sbp.tile([P, 1], F32, tag="rstd", name=f"rstd{mt}")
            nc.scalar.activation(rstd[:], rv[:], AF.Sqrt)
            yn = sbp.tile([P, E], F32, tag="yn", name=f"yn{mt}")
            nc.vector.tensor_scalar(yn[:], x[:], nm[:], rstd[:], op0=ALU.add, op1=ALU.mult)
            yg = sbp.tile([P, E], F32, tag="yg", name=f"yg{mt}")
            nc.vector.tensor_tensor(yg[:], yn[:], gam_bc[:], op=ALU.mult)
            yb = sbp.tile([P, E], F32, tag="yb", name=f"yb{mt}")
            nc.vector.tensor_tensor(yb[:], yg[:], bet_bc[:], op=ALU.add)
            nc.sync.dma_start(y_d.ap().rearrange("(m p) e -> p m e", p=P)[:, mt, :], yb[:])

    nc.compile()
    return nc


def _get(name, skv_c=None):
    key = (name, skv_c)
    if key not in _CACHE:
        _CACHE[key] = _build_phase_a(skv_c) if name == "a" else _build_phase_b()
    return _CACHE[key]


def _to_jslices(x):
    """[E, N] -> [P, NE, N] with j-slice (e // 128) as a free dim."""
    e, n = x.shape
    return np.ascontiguousarray(x.reshape(NE, P, n).transpose(1, 0, 2))


def kernel(query, key_value, key_value_mask, Wq, bq, Wk, bk, Wv, bv, Wo, bo,
           ln_gamma, ln_beta):
    f = lambda a: np.ascontiguousarray(np.asarray(a, dtype=np.float32))
    f8 = lambda a: np.ascontiguousarray(np.asarray(a, dtype=np.float32).astype(NPF8))
    query, key_value = f(query), f(key_value)
    Wq, Wk, Wv, Wo = f(Wq), f(Wk), f(Wv), f(Wo)
    bq, bk, bv, bo = f(bq), f(bk), f(bv), f(bo)
    ln_gamma, ln_beta = f(ln_gamma), f(ln_beta)
    maskb = np.asarray(key_value_mask) != 0

    # host-side kv compaction (pure gather + zero pad, shared per batch)
    pops = [int(maskb[b].sum()) for b in range(B)]
    skv_c = max(256, int(np.ceil(max(max(pops), 1) / 256.0)) * 256)
    skv_c = min(skv_c, SKV if SKV % 256 == 0 else SKV)
    NT = skv_c // P
    kvT8s, mask01s = [], []
    for b in range(B):
        idx = np.flatnonzero(maskb[b])
        kvc = np.zeros((skv_c, E), np.float32)
        kvc[:len(idx)] = key_value[b][idx]
        mc = np.zeros((skv_c,), np.float32)
        mc[:len(idx)] = 1.0
        kvT8s.append(f8(_to_jslices(kvc.T)))
        mask01s.append(f(mc.reshape(NT, P).T))

    def shufw(w):
        # [256 out-dims, E] -> lhsT j-slices [P, NE, HD]
        return f8(_to_jslices(w.T))

    nc_a = _get("a", skv_c)
    in_maps_a = []
    qT8s = [f8(_to_jslices(query[b].T)) for b in range(B)]
    for c in range(8):
        b, hg = c // 4, c % 4
        sl = slice(hg * HD, (hg + 1) * HD)
        in_maps_a.append({
            "qT8": qT8s[b],
            "kvT8": kvT8s[b],
            "wq8": shufw(Wq[sl]),
            "wk8": shufw(Wk[sl]),
            "wv8": shufw(Wv[sl]),
            "bq": f(bq[sl].reshape(2, P).T),
            "bk": f(bk[sl].reshape(2, P).T),
            "bv": bv[sl].reshape(1, HD),
            "mask01": mask01s[b],
        })
    res_a = run_bass_kernel_spmd(nc_a, in_maps_a, core_ids=list(range(8)))
    # gather: per batch, stack head-group slabs into the full [E, SQ] fp8 attnT
    attnT = [np.concatenate([res_a.results[b * 4 + hg]["attnT8"] for hg in range(4)],
                            axis=0) for b in range(B)]

    nc_b = _get("b")
    wo8 = f8(_to_jslices(Wo.T * WSC))
    bo512 = f(bo * (ASC * WSC)).reshape(1, E)
    gam_r = ln_gamma.reshape(1, E)
    bet_r = ln_beta.reshape(1, E)
    in_maps_b = []
    for c in range(8):
        b, j = c // 4, c % 4
        rs = slice(j * 256, (j + 1) * 256)
        in_maps_b.append({
            "aT8": np.ascontiguousarray(_to_jslices(attnT[b])[:, :, rs]),
            "wo8": wo8,
            "qn": f(query[b, rs, :]),
            "bo512": bo512,
            "gam": gam_r,
            "bet": bet_r,
        })
    res_b = run_bass_kernel_spmd(nc_b, in_maps_b, core_ids=list(range(8)))
    out = np.empty((B, SQ, E), np.float32)
    for c in range(8):
        b, j = c // 4, c % 4
        out[b, j * 256:(j + 1) * 256, :] = res_b.results[c]["y"]
    return out
